# revision 26
# baseline (speedup 1.0000x reference)
"""Expert-parallel MoE (top-1 routing) on 8 TRN2 NeuronCores.

Strategy (per sharding hint): one expert per core. The host computes the
top-1 gate assignment (tiny [N,8] matmul in fp64, matching the fp32
reference argmax) only to *dispatch* tokens: tokens are gathered per
expert, transposed to feature-major [D, C] and zero-padded to a common
capacity C so all 8 cores run one SPMD program. Each core then:
  - recomputes gate logits/softmax for its tokens on-device to get the
    chosen-prob scale row and the per-expert prob sums that feed the
    balancing loss (values only - no argmax - so bf16 matmul precision
    is sufficient),
  - runs its expert's FFN in bf16: h = gelu(x @ w1 + b1) [feature-major,
    so both matmuls consume the weights in natural layout as the
    stationary operand and no transposes are needed],
    y = (h @ w2 + b2) * chosen_prob, with b2 folded in as a rank-1 (K=1)
    PSUM accumulation,
  - writes y [D, C] and probsum [8].
The host scatters the per-expert outputs back to token order and reduces
the 8x8 probsums into the scalar balancing loss.

Inputs are shipped as a handful of large DMAs (HWDGE issue is serial per
instruction, so many small DMAs would stall the head of the kernel).
"""

import math

import numpy as np
import ml_dtypes

import concourse.bass as bass
import concourse.mybir as mybir
import concourse.tile as tile
from concourse.bass_utils import run_bass_kernel_spmd

P = 128
D = 768
F = 3072
E = 8
KD = D // P   # 6
KF = F // P   # 24
BF16 = mybir.dt.bfloat16
F32 = mybir.dt.float32
AF = mybir.ActivationFunctionType


def _packs(C):
    """Const-pack layouts. f32 pack columns: b1m [0:24], mask [24:24+TT],
    ident [24+TT:+128], ones-row region [+128] (row 0 is 1.0 - used as a
    [1,128] ones row). bf16 pack: b2 row [0:768], ones row [768:768+C],
    gwT [768+C:+48]."""
    TT = -(-C // P)
    o_b1, o_mask, o_id, o_ones = 0, 24, 24 + TT, 24 + TT + 128
    nf32 = o_ones + 128
    o_b2, o_or, o_gw = 0, D, D + C
    nbf = o_gw + KD * E
    return (o_b1, o_mask, o_id, o_ones, nf32), (o_b2, o_or, o_gw, nbf)


def _split_multi_waits(nc, maxw=1):
    """This container's walrus rejects >1 semaphore wait per instruction
    ("Too many sync wait commands" on the Tile tail drain). Move extra
    waits onto preceding EventSemaphore ops on the same engine; engine
    queues execute in order so blocking semantics are preserved."""
    for f in nc.m.functions:
        for bb in f.blocks:
            out = []
            changed = False
            for inst in bb.instructions:
                si = inst.sync_info
                if si is not None and si.on_wait and len(si.on_wait) > maxw:
                    waits = list(si.on_wait)
                    for j, w in enumerate(waits[:-maxw]):
                        nop = mybir.InstEventSemaphore(
                            name=f"Wsplit-{inst.name}-{j}", ins=[], outs=[])
                        nop.engine = inst.engine
                        nop.sync_info = mybir.SyncInfo(on_wait=[w], on_update=[])
                        out.append(nop)
                    si.on_wait = waits[-maxw:]
                    changed = True
                out.append(inst)
            if changed:
                bb.instructions = out


def _chunks(C):
    out = []
    n0 = 0
    while n0 < C:
        out.append((n0, min(512, C - n0)))
        n0 += 512
    return out


def build_nc(C):
    TT = -(-C // P)
    (o_b1, o_mask, o_id, o_ones, nf32), (o_b2, o_or, o_gw, nbf) = _packs(C)
    nc = bass.Bass()

    xTb = nc.declare_dram_parameter("xTb", [D, C], BF16, isOutput=False)
    w1 = nc.declare_dram_parameter("w1", [D, F], BF16, isOutput=False)
    w2 = nc.declare_dram_parameter("w2", [F, D], BF16, isOutput=False)
    pkf = nc.declare_dram_parameter("pkf", [P, nf32], F32, isOutput=False)
    pkb = nc.declare_dram_parameter("pkb", [P, nbf], BF16, isOutput=False)

    out = nc.declare_dram_parameter("out", [D, C], F32, isOutput=True)
    stats = nc.declare_dram_parameter("stats", [E, 1], F32, isOutput=True)

    xTb_t = xTb.rearrange("(k p) c -> p k c", p=P)
    w1_t = w1.rearrange("(k p) f -> p k f", p=P)
    w2_t = w2.rearrange("(k p) d -> p k d", p=P)

    with tile.TileContext(nc) as tc:
        with (
            tc.tile_pool(name="const", bufs=1) as const,
            tc.tile_pool(name="xin", bufs=1) as xin,
            tc.tile_pool(name="wts", bufs=1) as wts,
            tc.tile_pool(name="hbuf", bufs=1) as hbuf,
            tc.tile_pool(name="gate", bufs=2) as gate,
            tc.tile_pool(name="evict", bufs=2) as evict,
        ):
            # ---- inputs ----
            # The DMA wire is serial, so ship in consumption order: gate
            # inputs first, then w1 one k-slice at a time (each slice
            # unblocks one matmul in every open h accumulation group),
            # then w2 (needed only once h is done).
            pkb_sb = const.tile([P, nbf], BF16, tag="pkb", name="pkb")
            nc.sync.dma_start(pkb_sb[:], pkb[:, :])
            xb_all = xin.tile([P, KD, C], BF16, tag="xb", name="xb")
            nc.sync.dma_start(xb_all[:, 0:2, :], xTb_t[:, 0:2, :])
            nc.sync.dma_start(xb_all[:, 2:KD, :], xTb_t[:, 2:KD, :])
            pkf_sb = const.tile([P, nf32], F32, tag="pkf", name="pkf")
            nc.sync.dma_start(pkf_sb[:], pkf[:, :])
            w1_all = wts.tile([P, KD, F], BF16, tag="w1", name="w1")
            for k in range(KD):
                nc.sync.dma_start(w1_all[:, k:k + 1, :], w1_t[:, k:k + 1, :])
            w2_all = wts.tile([P, KF, D], BF16, tag="w2", name="w2")
            nc.sync.dma_start(w2_all[:, 0:12, :], w2_t[:, 0:12, :])
            nc.sync.dma_start(w2_all[:, 12:24, :], w2_t[:, 12:24, :])

            xb_sb = [xb_all[:, k, :] for k in range(KD)]
            w1_sb = [w1_all[:, k, :] for k in range(KD)]
            w2_sb = [w2_all[:, k, :] for k in range(KF)]
            gw_sb = [pkb_sb[:, o_gw + k * E:o_gw + (k + 1) * E] for k in range(KD)]
            b1_sb = pkf_sb[:, o_b1:o_b1 + 24]
            mask_sb = pkf_sb[:, o_mask:o_mask + TT]
            ident_sb = pkf_sb[:, o_id:o_id + P]
            ones1x128_sb = pkf_sb[0:1, o_ones:o_ones + P]
            b2_sb = pkb_sb[0:1, o_b2:o_b2 + D]
            onesrow_sb = pkb_sb[0:1, o_or:o_or + C]

            # ---- gate pass, token-major [128 tokens, 8 experts], bf16 mm ----
            # chosen prob = 1/sum(exp(l - max)); transposed to a row with a
            # fp32 matmul against the identity, then broadcast to all 128
            # partitions with a ones (x) row matmul.
            pb_sb = gate.tile([P, C], F32, tag="pb", name="pb")
            ctx_gate = tc.tile_pool(name="psg", bufs=1, space="PSUM")
            psg = ctx_gate.__enter__()
            st_ps = psg.tile([E, 1], F32, tag="st", name="st")
            for tt in range(TT):
                w = min(P, C - tt * P)
                sl = slice(tt * P, tt * P + w)
                lg = psg.tile([P, E], F32, tag="lg", name="lg")
                for k in range(KD):
                    nc.tensor.matmul(lg[:w], xb_sb[k][:, sl], gw_sb[k],
                                     start=(k == 0), stop=(k == KD - 1))
                mx = gate.tile([P, 1], F32, tag="mx", name="mx")
                nc.vector.reduce_max(mx[:w], lg[:w], axis=mybir.AxisListType.X)
                negm = gate.tile([P, 1], F32, tag="negm", name="negm")
                nc.vector.tensor_scalar_mul(negm[:w], mx[:w], -1.0)
                pe_t = gate.tile([P, E], F32, tag="pe", name="pe")
                sume = gate.tile([P, 1], F32, tag="sume", name="sume")
                nc.scalar.activation(pe_t[:w], lg[:w], AF.Exp, bias=negm[:w],
                                     accum_out=sume[:w])
                rc = gate.tile([P, 1], F32, tag="rc", name="rc")
                nc.vector.reciprocal(rc[:w], sume[:w])
                probs = gate.tile([P, E], F32, tag="probs", name="probs")
                nc.vector.tensor_scalar_mul(probs[:w], pe_t[:w], rc[:w])
                nc.tensor.matmul(st_ps[:], probs[:w], mask_sb[:w, tt:tt + 1],
                                 start=(tt == 0), stop=(tt == TT - 1))
                ct_ps = psg.tile([1, P], F32, tag="ct", name="ct")
                nc.tensor.matmul(ct_ps[:], rc[:w], ident_sb[:w, :],
                                 start=True, stop=True)
                ct_sb = gate.tile([1, P], F32, tag="ctsb", name="ctsb")
                nc.vector.tensor_copy(ct_sb[:], ct_ps[:])
                pb_ps = psg.tile([P, P], F32, tag="pbps", name="pbps")
                nc.tensor.matmul(pb_ps[:], ones1x128_sb, ct_sb[:],
                                 start=True, stop=True)
                nc.vector.tensor_copy(pb_sb[:, sl], pb_ps[:, :w])
            stats_sb = gate.tile([E, 1], F32, tag="stacc", name="stacc")
            nc.vector.tensor_copy(stats_sb[:], st_ps[:])
            nc.sync.dma_start(stats[:, :], stats_sb[:])
            ctx_gate.__exit__(None, None, None)
            ctx_mm = tc.tile_pool(name="psmm", bufs=8, space="PSUM")
            psmm = ctx_mm.__enter__()

            # ---- FFN (bf16): h = gelu(w1.T @ x + b1); y = w2.T @ h + b2 ----
            h_sb = [hbuf.tile([P, C], BF16, tag=f"h{ft}", name=f"h{ft}")
                    for ft in range(KF)]
            for (n0, nsz) in _chunks(C):
                cs = slice(n0, n0 + nsz)
                for ft in range(KF):
                    hp = psmm.tile([P, 512], F32, tag="mm0", name="mm0")
                    for k in range(KD):
                        nc.tensor.matmul(hp[:, :nsz],
                                         w1_sb[k][:, ft * P:(ft + 1) * P],
                                         xb_sb[k][:, cs],
                                         start=(k == 0), stop=(k == KD - 1))
                    nc.scalar.activation(h_sb[ft][:, cs], hp[:, :nsz], AF.Gelu,
                                         bias=b1_sb[:, ft:ft + 1])
            for dt in range(KD):
                ot = evict.tile([P, C], F32, tag="ot", name="ot")
                for (n0, nsz) in _chunks(C):
                    cs = slice(n0, n0 + nsz)
                    yp = psmm.tile([P, 512], F32, tag="mm0", name="mm0")
                    for k in range(KF):
                        nc.tensor.matmul(yp[:, :nsz],
                                         w2_sb[k][:, dt * P:(dt + 1) * P],
                                         h_sb[k][:, cs],
                                         start=(k == 0), stop=False)
                    nc.tensor.matmul(yp[:, :nsz],
                                     b2_sb[0:1, dt * P:(dt + 1) * P],
                                     onesrow_sb[0:1, cs],
                                     start=False, stop=True)
                    nc.vector.tensor_mul(ot[:, cs], yp[:, :nsz], pb_sb[:, cs])
                    nc.sync.dma_start(out[dt * P:(dt + 1) * P, cs], ot[:, cs])
            ctx_mm.__exit__(None, None, None)

    _split_multi_waits(nc)
    return nc


_NC_CACHE = {}


def prepare(x, attention_mask, gate_w, w1, b1, w2, b2):
    """Host-side dispatch: returns (nc, in_maps, idx_per_core, counts, C)."""
    x = np.asarray(x, np.float32)
    gate_w = np.asarray(gate_w, np.float32)
    w1 = np.asarray(w1, np.float32)
    b1 = np.asarray(b1, np.float32)
    w2 = np.asarray(w2, np.float32)
    b2 = np.asarray(b2, np.float32)

    B, S, _ = x.shape
    N = B * S
    xf = x.reshape(N, D)

    # Top-1 expert per token (fp64 logits so the argmax matches the fp32
    # reference on near-ties).
    logits = xf.astype(np.float64) @ gate_w.T.astype(np.float64)
    gate_idx = np.argmax(logits, axis=1)
    counts = np.bincount(gate_idx, minlength=E)
    order = np.argsort(gate_idx, kind="stable")
    bounds = np.concatenate([[0], np.cumsum(counts)])
    C = max(P, int(math.ceil(counts.max() / 64) * 64))
    TT = -(-C // P)
    (o_b1, o_mask, o_id, o_ones, nf32), (o_b2, o_or, o_gw, nbf) = _packs(C)

    if C not in _NC_CACHE:
        _NC_CACHE[C] = build_nc(C)
    nc = _NC_CACHE[C]

    gwT = np.ascontiguousarray(gate_w.T)  # [D, E]

    in_maps = []
    idx_per_core = []
    for c in range(E):
        idx = order[bounds[c]:bounds[c + 1]]
        idx_per_core.append(idx)
        n_c = len(idx)
        xT = np.zeros((D, C), np.float32)
        xT[:, :n_c] = xf[idx].T

        pkf_np = np.zeros((P, nf32), np.float32)
        pkf_np[:, o_b1:o_b1 + KF] = b1[c].reshape(KF, P).T
        mask = np.zeros(TT * P, np.float32)
        mask[:n_c] = 1.0
        pkf_np[:, o_mask:o_mask + TT] = mask.reshape(TT, P).T
        pkf_np[:, o_id:o_id + P] = np.eye(P, dtype=np.float32)
        pkf_np[0, o_ones:o_ones + P] = 1.0

        pkb_np = np.zeros((P, nbf), ml_dtypes.bfloat16)
        pkb_np[0, o_b2:o_b2 + D] = b2[c].astype(ml_dtypes.bfloat16)
        pkb_np[0, o_or:o_or + C] = ml_dtypes.bfloat16(1.0)
        pkb_np[:, o_gw:o_gw + KD * E] = gwT.reshape(KD, P, E).transpose(
            1, 0, 2).reshape(P, KD * E).astype(ml_dtypes.bfloat16)

        in_maps.append({
            "xTb": xT.astype(ml_dtypes.bfloat16),
            "w1": w1[c].astype(ml_dtypes.bfloat16),
            "w2": w2[c].astype(ml_dtypes.bfloat16),
            "pkf": pkf_np,
            "pkb": pkb_np,
        })
    return nc, in_maps, idx_per_core, counts, C


def kernel(x, attention_mask, gate_w, w1, b1, w2, b2):
    x = np.asarray(x, np.float32)
    B, S, _ = x.shape
    N = B * S
    nc, in_maps, idx_per_core, counts, C = prepare(
        x, attention_mask, gate_w, w1, b1, w2, b2)

    res = run_bass_kernel_spmd(nc, in_maps, core_ids=list(range(E)))

    out_flat = np.zeros((N, D), np.float32)
    probsum = np.zeros(E, np.float64)
    for c in range(E):
        idx = idx_per_core[c]
        out_flat[idx] = res.results[c]["out"][:, :len(idx)].T
        probsum += res.results[c]["stats"].reshape(E).astype(np.float64)

    Pm = (probsum / N).astype(np.float32)
    fm = (counts / N).astype(np.float32)
    balance_loss = np.float32(E * np.sum(Pm * fm))
    gate_load = counts.astype(np.int32)
    return out_flat.reshape(B, S, D), balance_loss, gate_load


# revision 27
# speedup vs baseline: 1.8347x; 1.8347x over previous
"""Expert-parallel MoE (top-1 routing) on 8 TRN2 NeuronCores.

Strategy (per sharding hint): one expert per core. The host computes the
top-1 gate assignment (tiny [N,8] matmul in fp64, matching the fp32
reference argmax) only to *dispatch* tokens: tokens are gathered per
expert, transposed to feature-major [D, C] and zero-padded to a common
capacity C so all 8 cores run one SPMD program. Each core then:
  - recomputes gate logits/softmax for its tokens on-device to get the
    chosen-prob scale row and the per-expert prob sums that feed the
    balancing loss (values only - no argmax - so bf16 matmul precision
    is sufficient),
  - runs its expert's FFN in bf16: h = gelu(x @ w1 + b1) [feature-major,
    so both matmuls consume the weights in natural layout as the
    stationary operand and no transposes are needed],
    y = (h @ w2 + b2) * chosen_prob, with b2 folded in as a rank-1 (K=1)
    PSUM accumulation,
  - writes y [D, C] and probsum [8].
The host scatters the per-expert outputs back to token order and reduces
the 8x8 probsums into the scalar balancing loss.

Inputs are shipped as a handful of large DMAs (HWDGE issue is serial per
instruction, so many small DMAs would stall the head of the kernel).
"""

import math

import numpy as np
import ml_dtypes

import concourse.bass as bass
import concourse.mybir as mybir
import concourse.tile as tile
from concourse.bass_utils import run_bass_kernel_spmd

P = 128
D = 768
F = 3072
E = 8
KD = D // P   # 6
KF = F // P   # 24
BF16 = mybir.dt.bfloat16
F32 = mybir.dt.float32
AF = mybir.ActivationFunctionType


def _packs(C):
    """Const-pack layouts. f32 pack columns: b1m [0:24], mask [24:24+TT],
    ident [24+TT:+128], ones-row region [+128] (row 0 is 1.0 - used as a
    [1,128] ones row). bf16 pack: b2 row [0:768], ones row [768:768+C],
    gwT [768+C:+48]."""
    TT = -(-C // P)
    o_b1, o_mask, o_id, o_ones = 0, 24, 24 + TT, 24 + TT + 128
    nf32 = o_ones + 128
    o_b2, o_or, o_gw = 0, D, D + C
    nbf = o_gw + KD * E
    return (o_b1, o_mask, o_id, o_ones, nf32), (o_b2, o_or, o_gw, nbf)


def _split_multi_waits(nc, maxw=1):
    """This container's walrus rejects >1 semaphore wait per instruction
    ("Too many sync wait commands" on the Tile tail drain). Move extra
    waits onto preceding EventSemaphore ops on the same engine; engine
    queues execute in order so blocking semantics are preserved."""
    for f in nc.m.functions:
        for bb in f.blocks:
            out = []
            changed = False
            for inst in bb.instructions:
                si = inst.sync_info
                if si is not None and si.on_wait and len(si.on_wait) > maxw:
                    waits = list(si.on_wait)
                    for j, w in enumerate(waits[:-maxw]):
                        nop = mybir.InstEventSemaphore(
                            name=f"Wsplit-{inst.name}-{j}", ins=[], outs=[])
                        nop.engine = inst.engine
                        nop.sync_info = mybir.SyncInfo(on_wait=[w], on_update=[])
                        out.append(nop)
                    si.on_wait = waits[-maxw:]
                    changed = True
                out.append(inst)
            if changed:
                bb.instructions = out


def _chunks(C):
    out = []
    n0 = 0
    while n0 < C:
        out.append((n0, min(512, C - n0)))
        n0 += 512
    return out


def build_nc(C):
    TT = -(-C // P)
    (o_b1, o_mask, o_id, o_ones, nf32), (o_b2, o_or, o_gw, nbf) = _packs(C)
    nc = bass.Bass()

    xTb = nc.declare_dram_parameter("xTb", [D, C], BF16, isOutput=False)
    w1 = nc.declare_dram_parameter("w1", [D, F], BF16, isOutput=False)
    w2 = nc.declare_dram_parameter("w2", [F, D], BF16, isOutput=False)
    pkf = nc.declare_dram_parameter("pkf", [P, nf32], F32, isOutput=False)
    pkb = nc.declare_dram_parameter("pkb", [P, nbf], BF16, isOutput=False)

    out = nc.declare_dram_parameter("out", [D, C], F32, isOutput=True)
    stats = nc.declare_dram_parameter("stats", [E, 1], F32, isOutput=True)

    xTb_t = xTb.rearrange("(k p) c -> p k c", p=P)
    w1_t = w1.rearrange("(k p) f -> p k f", p=P)
    w2_t = w2.rearrange("(k p) d -> p k d", p=P)

    with tile.TileContext(nc) as tc:
        with (
            tc.tile_pool(name="const", bufs=1) as const,
            tc.tile_pool(name="xin", bufs=1) as xin,
            tc.tile_pool(name="wts", bufs=1) as wts,
            tc.tile_pool(name="hbuf", bufs=1) as hbuf,
            tc.tile_pool(name="gate", bufs=2) as gate,
            tc.tile_pool(name="evict", bufs=2) as evict,
        ):
            # ---- inputs ----
            # The DMA wire is serial, so ship in consumption order: gate
            # inputs first, then w1 one k-slice at a time (each slice
            # unblocks one matmul in every open h accumulation group),
            # then w2 (needed only once h is done).
            xb_all = xin.tile([P, KD, C], BF16, tag="xb", name="xb")
            nc.sync.dma_start(xb_all[:, 0:2, :], xTb_t[:, 0:2, :])
            pkb_sb = const.tile([P, nbf], BF16, tag="pkb", name="pkb")
            nc.sync.dma_start(pkb_sb[:], pkb[:, :])
            nc.sync.dma_start(xb_all[:, 2:KD, :], xTb_t[:, 2:KD, :])
            pkf_sb = const.tile([P, nf32], F32, tag="pkf", name="pkf")
            nc.sync.dma_start(pkf_sb[:], pkf[:, :])
            w1_all = wts.tile([P, KD, F], BF16, tag="w1", name="w1")
            for k in range(KD):
                nc.sync.dma_start(w1_all[:, k:k + 1, :], w1_t[:, k:k + 1, :])
            w2_all = wts.tile([P, KF, D], BF16, tag="w2", name="w2")
            nc.sync.dma_start(w2_all[:, 0:12, :], w2_t[:, 0:12, :])
            nc.sync.dma_start(w2_all[:, 12:24, :], w2_t[:, 12:24, :])

            xb_sb = [xb_all[:, k, :] for k in range(KD)]
            w1_sb = [w1_all[:, k, :] for k in range(KD)]
            w2_sb = [w2_all[:, k, :] for k in range(KF)]
            gw_sb = [pkb_sb[:, o_gw + k * E:o_gw + (k + 1) * E] for k in range(KD)]
            b1_sb = pkf_sb[:, o_b1:o_b1 + 24]
            mask_sb = pkf_sb[:, o_mask:o_mask + TT]
            ident_sb = pkf_sb[:, o_id:o_id + P]
            ones1x128_sb = pkf_sb[0:1, o_ones:o_ones + P]
            b2_sb = pkb_sb[0:1, o_b2:o_b2 + D]
            onesrow_sb = pkb_sb[0:1, o_or:o_or + C]

            # ---- gate pass, token-major [128 tokens, 8 experts], bf16 mm ----
            # chosen prob = 1/sum(exp(l - max)); transposed to a row with a
            # fp32 matmul against the identity, then broadcast to all 128
            # partitions with a ones (x) row matmul.
            pb_sb = gate.tile([P, C], F32, tag="pb", name="pb")
            ctx_gate = tc.tile_pool(name="psg", bufs=1, space="PSUM")
            psg = ctx_gate.__enter__()
            st_ps = psg.tile([E, 1], F32, tag="st", name="st")
            for tt in range(TT):
                w = min(P, C - tt * P)
                sl = slice(tt * P, tt * P + w)
                lg = psg.tile([P, E], F32, tag="lg", name="lg")
                for k in range(KD):
                    nc.tensor.matmul(lg[:w], xb_sb[k][:, sl], gw_sb[k],
                                     start=(k == 0), stop=(k == KD - 1))
                mx = gate.tile([P, 1], F32, tag="mx", name="mx")
                nc.vector.reduce_max(mx[:w], lg[:w], axis=mybir.AxisListType.X)
                negm = gate.tile([P, 1], F32, tag="negm", name="negm")
                nc.vector.tensor_scalar_mul(negm[:w], mx[:w], -1.0)
                pe_t = gate.tile([P, E], F32, tag="pe", name="pe")
                sume = gate.tile([P, 1], F32, tag="sume", name="sume")
                nc.scalar.activation(pe_t[:w], lg[:w], AF.Exp, bias=negm[:w],
                                     accum_out=sume[:w])
                rc = gate.tile([P, 1], F32, tag="rc", name="rc")
                nc.vector.reciprocal(rc[:w], sume[:w])
                probs = gate.tile([P, E], F32, tag="probs", name="probs")
                nc.vector.tensor_scalar_mul(probs[:w], pe_t[:w], rc[:w])
                nc.tensor.matmul(st_ps[:], probs[:w], mask_sb[:w, tt:tt + 1],
                                 start=(tt == 0), stop=(tt == TT - 1))
                ct_ps = psg.tile([1, P], F32, tag="ct", name="ct")
                nc.tensor.matmul(ct_ps[:], rc[:w], ident_sb[:w, :],
                                 start=True, stop=True)
                ct_sb = gate.tile([1, P], F32, tag="ctsb", name="ctsb")
                nc.vector.tensor_copy(ct_sb[:], ct_ps[:])
                pb_ps = psg.tile([P, P], F32, tag="pbps", name="pbps")
                nc.tensor.matmul(pb_ps[:], ones1x128_sb, ct_sb[:],
                                 start=True, stop=True)
                nc.vector.tensor_copy(pb_sb[:, sl], pb_ps[:, :w])
            stats_sb = gate.tile([E, 1], F32, tag="stacc", name="stacc")
            nc.vector.tensor_copy(stats_sb[:], st_ps[:])
            nc.sync.dma_start(stats[:, :], stats_sb[:])
            ctx_gate.__exit__(None, None, None)
            ctx_mm = tc.tile_pool(name="psmm", bufs=8, space="PSUM")
            psmm = ctx_mm.__enter__()

            # ---- FFN (bf16): h = gelu(w1.T @ x + b1); y = w2.T @ h + b2 ----
            h_sb = [hbuf.tile([P, C], BF16, tag=f"h{ft}", name=f"h{ft}")
                    for ft in range(KF)]
            for (n0, nsz) in _chunks(C):
                cs = slice(n0, n0 + nsz)
                for ft in range(KF):
                    hp = psmm.tile([P, 512], F32, tag="mm0", name="mm0")
                    for k in range(KD):
                        nc.tensor.matmul(hp[:, :nsz],
                                         w1_sb[k][:, ft * P:(ft + 1) * P],
                                         xb_sb[k][:, cs],
                                         start=(k == 0), stop=(k == KD - 1))
                    nc.scalar.activation(h_sb[ft][:, cs], hp[:, :nsz], AF.Gelu,
                                         bias=b1_sb[:, ft:ft + 1])
            for dt in range(KD):
                ot = evict.tile([P, C], F32, tag="ot", name="ot")
                for (n0, nsz) in _chunks(C):
                    cs = slice(n0, n0 + nsz)
                    yp = psmm.tile([P, 512], F32, tag="mm0", name="mm0")
                    for k in range(KF):
                        nc.tensor.matmul(yp[:, :nsz],
                                         w2_sb[k][:, dt * P:(dt + 1) * P],
                                         h_sb[k][:, cs],
                                         start=(k == 0), stop=False)
                    nc.tensor.matmul(yp[:, :nsz],
                                     b2_sb[0:1, dt * P:(dt + 1) * P],
                                     onesrow_sb[0:1, cs],
                                     start=False, stop=True)
                    nc.vector.tensor_mul(ot[:, cs], yp[:, :nsz], pb_sb[:, cs])
                    nc.sync.dma_start(out[dt * P:(dt + 1) * P, cs], ot[:, cs])
            ctx_mm.__exit__(None, None, None)

    _split_multi_waits(nc)
    return nc


_NC_CACHE = {}


def prepare(x, attention_mask, gate_w, w1, b1, w2, b2):
    """Host-side dispatch: returns (nc, in_maps, idx_per_core, counts, C)."""
    x = np.asarray(x, np.float32)
    gate_w = np.asarray(gate_w, np.float32)
    w1 = np.asarray(w1, np.float32)
    b1 = np.asarray(b1, np.float32)
    w2 = np.asarray(w2, np.float32)
    b2 = np.asarray(b2, np.float32)

    B, S, _ = x.shape
    N = B * S
    xf = x.reshape(N, D)

    # Top-1 expert per token (fp64 logits so the argmax matches the fp32
    # reference on near-ties).
    logits = xf.astype(np.float64) @ gate_w.T.astype(np.float64)
    gate_idx = np.argmax(logits, axis=1)
    counts = np.bincount(gate_idx, minlength=E)
    order = np.argsort(gate_idx, kind="stable")
    bounds = np.concatenate([[0], np.cumsum(counts)])
    C = max(P, int(math.ceil(counts.max() / 64) * 64))
    TT = -(-C // P)
    (o_b1, o_mask, o_id, o_ones, nf32), (o_b2, o_or, o_gw, nbf) = _packs(C)

    if C not in _NC_CACHE:
        _NC_CACHE[C] = build_nc(C)
    nc = _NC_CACHE[C]

    gwT = np.ascontiguousarray(gate_w.T)  # [D, E]

    in_maps = []
    idx_per_core = []
    for c in range(E):
        idx = order[bounds[c]:bounds[c + 1]]
        idx_per_core.append(idx)
        n_c = len(idx)
        xT = np.zeros((D, C), np.float32)
        xT[:, :n_c] = xf[idx].T

        pkf_np = np.zeros((P, nf32), np.float32)
        pkf_np[:, o_b1:o_b1 + KF] = b1[c].reshape(KF, P).T
        mask = np.zeros(TT * P, np.float32)
        mask[:n_c] = 1.0
        pkf_np[:, o_mask:o_mask + TT] = mask.reshape(TT, P).T
        pkf_np[:, o_id:o_id + P] = np.eye(P, dtype=np.float32)
        pkf_np[0, o_ones:o_ones + P] = 1.0

        pkb_np = np.zeros((P, nbf), ml_dtypes.bfloat16)
        pkb_np[0, o_b2:o_b2 + D] = b2[c].astype(ml_dtypes.bfloat16)
        pkb_np[0, o_or:o_or + C] = ml_dtypes.bfloat16(1.0)
        pkb_np[:, o_gw:o_gw + KD * E] = gwT.reshape(KD, P, E).transpose(
            1, 0, 2).reshape(P, KD * E).astype(ml_dtypes.bfloat16)

        in_maps.append({
            "xTb": xT.astype(ml_dtypes.bfloat16),
            "w1": w1[c].astype(ml_dtypes.bfloat16),
            "w2": w2[c].astype(ml_dtypes.bfloat16),
            "pkf": pkf_np,
            "pkb": pkb_np,
        })
    return nc, in_maps, idx_per_core, counts, C


def kernel(x, attention_mask, gate_w, w1, b1, w2, b2):
    x = np.asarray(x, np.float32)
    B, S, _ = x.shape
    N = B * S
    nc, in_maps, idx_per_core, counts, C = prepare(
        x, attention_mask, gate_w, w1, b1, w2, b2)

    res = run_bass_kernel_spmd(nc, in_maps, core_ids=list(range(E)))

    out_flat = np.zeros((N, D), np.float32)
    probsum = np.zeros(E, np.float64)
    for c in range(E):
        idx = idx_per_core[c]
        out_flat[idx] = res.results[c]["out"][:, :len(idx)].T
        probsum += res.results[c]["stats"].reshape(E).astype(np.float64)

    Pm = (probsum / N).astype(np.float32)
    fm = (counts / N).astype(np.float32)
    balance_loss = np.float32(E * np.sum(Pm * fm))
    gate_load = counts.astype(np.int32)
    return out_flat.reshape(B, S, D), balance_loss, gate_load
